# revision 15
# baseline (speedup 1.0000x reference)
"""GraphSAGE (3-layer, mean-agg + BN + ReLU) SPMD kernel for trn2 NeuronCores.

Sharding: dst-node shards of n_nodes/n_cores per core. dma_gather uses int16
indices, so each source table is addressed through two PARITY VIEWS (even /
odd rows, via elem_step = 2 rows): view indices are src//2 < 25000, in int16
range. Per core, edges are sorted by (src_parity, dst_tile, src) and padded
into 128-edge chunks that are dst-tile pure and parity pure. Chunks are
gathered in large multi-chunk dma_gather calls (MC chunks per call) to
amortize the SWDGE per-call fixed cost.

Aggregation per chunk is a one-hot matmul accumulated in PSUM, feature-major:
    agg_T[din_blk, 128 dst] += gathered[128 e, din_blk].T @ S[128 e, 128 dst]

Layers 1/2 use the z-trick: z = h @ Wl computed per-shard node-major (mean
division commutes with Wl) and AllGathered as one large collective. bl biases
cancel under BN and are dropped. BN stats are feature-major bn_stats/bn_aggr,
combined across cores with a small AllGather and a vectorized cross-core tree
reduction.
"""
import numpy as np
import ml_dtypes
import concourse.bass as bass
import concourse.bacc as bacc
import concourse.tile as tile
from concourse import mybir
from concourse.masks import make_identity
from concourse.library_config import mlp

P = 128
F32 = mybir.dt.float32
BF16 = mybir.dt.bfloat16
I32 = mybir.dt.int32
I16 = mybir.dt.int16
MC = 8  # chunks per dma_gather call (1024 idxs — hard ucode limit)


# ---------------------------------------------------------------- host prep
def preprocess(edge_index, n_nodes, n_cores):
    src = np.asarray(edge_index[0], dtype=np.int64)
    dst = np.asarray(edge_index[1], dtype=np.int64)
    shard = n_nodes // n_cores
    ntiles = (shard + P - 1) // P

    deg = np.bincount(dst, minlength=n_nodes).astype(np.float32)
    rd_full = (1.0 / np.maximum(deg, 1.0)).astype(np.float32)

    core_of = dst // shard
    tile_of = (dst % shard) // P
    piece_of = src % 2          # parity view of the source table
    pidx = src // 2             # row index within the parity view
    assert pidx.max() < 32768

    # parity-major, then tile, then src: one gather stream per parity
    order = np.lexsort((src, tile_of, piece_of, core_of))
    pidx_s, dst_s = pidx[order], dst[order]

    key = (core_of[order] * 2 + piece_of[order]) * ntiles + tile_of[order]
    counts = np.bincount(key, minlength=n_cores * 2 * ntiles).reshape(
        n_cores, 2, ntiles)
    nch = np.ceil(counts / P).astype(np.int64).max(axis=0)  # [2, ntiles] shared
    nlo, nhi = int(nch[0].sum()), int(nch[1].sum())
    ntot = nlo + nhi
    lo0 = np.zeros(ntiles, dtype=np.int64)
    lo0[1:] = np.cumsum(nch[0])[:-1]
    hi0 = np.zeros(ntiles, dtype=np.int64)
    hi0[1:] = np.cumsum(nch[1])[:-1]

    starts = np.zeros(n_cores * 2 * ntiles + 1, dtype=np.int64)
    np.cumsum(counts.reshape(-1), out=starts[1:])

    pvec = np.arange(P)
    per_core = []
    for c in range(n_cores):
        idx16 = np.zeros((16, ntot * 8), dtype=np.int16)
        dstl = np.full((P, ntot), -1, dtype=np.int32)
        ci = 0
        for h in range(2):
            for t in range(ntiles):
                k = (c * 2 + h) * ntiles + t
                lo, hi = int(starts[k]), int(starts[k + 1])
                e_src = pidx_s[lo:hi]
                e_dst = dst_s[lo:hi] % shard - t * P
                for cc in range(int(nch[h, t])):
                    a = cc * P
                    sl_src = e_src[a:a + P]
                    sl_dst = e_dst[a:a + P]
                    m = len(sl_src)
                    if m > 0:
                        pv = pvec[:m]
                        idx16[pv % 16, ci * 8 + pv // 16] = sl_src.astype(np.int16)
                        dstl[:m, ci] = sl_dst.astype(np.int32)
                    ci += 1
        assert ci == ntot
        per_core.append({
            "idx16": np.tile(idx16, (8, 1)),
            "dstl": dstl,
            "rd": np.concatenate([
                rd_full[c * shard:(c + 1) * shard],
                np.ones(ntiles * P - shard, np.float32)]),
        })

    meta = {"n_nodes": n_nodes, "n_cores": n_cores, "shard": shard,
            "ntiles": ntiles, "nch": nch, "nlo": nlo, "nhi": nhi,
            "lo0": lo0, "hi0": hi0, "ntot": ntot}
    return meta, per_core


# ---------------------------------------------------------------- builder
def build_kernel(meta, dims, eps=1e-5):
    n_cores = meta["n_cores"]
    shard, ntiles, ntot = meta["shard"], meta["ntiles"], meta["ntot"]
    nch, nlo, nhi = meta["nch"], meta["nlo"], meta["nhi"]
    lo0, hi0 = meta["lo0"], meta["hi0"]
    n_nodes = meta["n_nodes"]
    d0, d1, d2, d3 = dims
    assert d0 == P
    nb1, nb2, nb3 = d1 // P, d2 // P, d3 // P

    nc = bacc.Bacc(debug=False, num_devices=n_cores)

    xg = nc.declare_dram_parameter("xg", [n_nodes, d0], BF16, isOutput=False)
    x_own_T = nc.declare_dram_parameter("x_own_T", [d0, shard], BF16, isOutput=False)
    idx16_d = nc.declare_dram_parameter("idx16", [P, ntot * 8], I16, isOutput=False)
    dstl_d = nc.declare_dram_parameter("dstl", [P, ntot], I32, isOutput=False)
    rd_d = nc.declare_dram_parameter("rd", [ntiles * P], F32, isOutput=False)
    Wl0 = nc.declare_dram_parameter("Wl0", [d0, d1], F32, isOutput=False)
    Wr0 = nc.declare_dram_parameter("Wr0", [d0, d1], BF16, isOutput=False)
    Wl1 = nc.declare_dram_parameter("Wl1", [d1, d2], BF16, isOutput=False)
    Wr1 = nc.declare_dram_parameter("Wr1", [d1, d2], BF16, isOutput=False)
    Wl2 = nc.declare_dram_parameter("Wl2", [d2, d3], BF16, isOutput=False)
    Wr2 = nc.declare_dram_parameter("Wr2", [d2, d3], BF16, isOutput=False)
    g_d = [nc.declare_dram_parameter(f"gn{i}", [dims[i + 1]], F32, isOutput=False) for i in range(3)]
    b_d = [nc.declare_dram_parameter(f"bn{i}", [dims[i + 1]], F32, isOutput=False) for i in range(3)]
    yout = nc.declare_dram_parameter("yout", [shard, d3], F32, isOutput=True)

    rg = [list(range(n_cores))]

    def tw(t):
        return min(P, shard - t * P)

    from contextlib import ExitStack
    with tile.TileContext(nc) as tc, ExitStack() as _st:
        pp = _st.enter_context(tc.tile_pool(name="persist", bufs=1))
        sp = _st.enter_context(tc.tile_pool(name="onehot", bufs=3))
        gplo = _st.enter_context(tc.tile_pool(name="glo", bufs=3))
        gphi = _st.enter_context(tc.tile_pool(name="ghi", bufs=2))
        wp = _st.enter_context(tc.tile_pool(name="work", bufs=3))
        smp = _st.enter_context(tc.tile_pool(name="small", bufs=4))
        psA = _st.enter_context(tc.tile_pool(name="psA", bufs=3, space="PSUM"))
        psB = _st.enter_context(tc.tile_pool(name="psB", bufs=3, space="PSUM"))
        psC = _st.enter_context(tc.tile_pool(name="psC", bufs=2, space="PSUM"))
        dp = _st.enter_context(tc.tile_pool(name="dram", bufs=1, space="DRAM"))
        hAB = _st.enter_context(tc.tile_pool(name="hAB", bufs=1))

        nc.gpsimd.load_library(mlp)

        idx_t = pp.tile([P, ntot * 8], I16)
        nc.gpsimd.dma_start(out=idx_t[:], in_=idx16_d[:])
        dstl_f = pp.tile([P, ntot], F32)
        dstl_i = wp.tile([P, ntot], I32, tag="dstli")
        nc.gpsimd.dma_start(out=dstl_i[:], in_=dstl_d[:])
        nc.vector.tensor_copy(out=dstl_f[:], in_=dstl_i[:])
        iota_f = pp.tile([P, P], F32)
        iota_i = wp.tile([P, P], I32, tag="iotai")
        nc.gpsimd.iota(iota_i[:], pattern=[[1, P]], base=0, channel_multiplier=0)
        nc.vector.tensor_copy(out=iota_f[:], in_=iota_i[:])
        ident = pp.tile([P, P], F32)
        make_identity(nc, ident[:])
        eps_t = pp.tile([P, 1], F32)
        nc.vector.memset(eps_t[:], float(eps))

        # bf16 weights (persist)
        wl1 = pp.tile([P, (d1 // P) * d2], BF16)
        nc.sync.dma_start(out=wl1[:].rearrange("p (k n) -> p k n", n=d2), in_=Wl1[:].rearrange("(k p) n -> p k n", p=P))
        wr1 = pp.tile([P, (d1 // P) * d2], BF16)
        nc.sync.dma_start(out=wr1[:].rearrange("p (k n) -> p k n", n=d2), in_=Wr1[:].rearrange("(k p) n -> p k n", p=P))
        wl2 = pp.tile([P, (d2 // P) * d3], BF16)
        nc.sync.dma_start(out=wl2[:].rearrange("p (k n) -> p k n", n=d3), in_=Wl2[:].rearrange("(k p) n -> p k n", p=P))
        wr2 = pp.tile([P, (d2 // P) * d3], BF16)
        nc.sync.dma_start(out=wr2[:].rearrange("p (k n) -> p k n", n=d3), in_=Wr2[:].rearrange("(k p) n -> p k n", p=P))

        # internal DRAM
        z1_sh = dp.tile([shard, d2], BF16)
        z1_full = dp.tile([n_cores * shard, d2], BF16)
        z2_sh = dp.tile([shard, d3], BF16)
        z2_full = dp.tile([n_cores * shard, d3], BF16)
        st_sh = [dp.tile([P, 2 * n], F32, tag=f"stsh{i}", name=f"stsh{i}") for i, n in enumerate((nb1, nb2, nb3))]
        st_full = [dp.tile([n_cores * P, 2 * n], F32, tag=f"stfl{i}", name=f"stfl{i}") for i, n in enumerate((nb1, nb2, nb3))]

        # ---------------- helpers
        def rd_bcast(t):
            rdb = smp.tile([P, P], F32, tag="rdb")
            nc.sync.dma_start(
                out=rdb[:], in_=rd_d[t * P:(t + 1) * P].partition_broadcast(P))
            return rdb

        def build_onehot(t):
            """S [P, nch_t, P] one-hot; cols = piece-A chunks of t, then B."""
            ncl, nchh = int(nch[0][t]), int(nch[1][t])
            nch_t = ncl + nchh
            S = sp.tile([P, nch_t, P], BF16, tag="S")
            for (base, cnt, off) in ((int(lo0[t]), ncl, 0),
                                     (nlo + int(hi0[t]), nchh, ncl)):
                if cnt == 0:
                    continue
                nc.vector.tensor_tensor(
                    out=S[:, off:off + cnt, :],
                    in0=dstl_f[:, base:base + cnt].unsqueeze(2).to_broadcast([P, cnt, P]),
                    in1=iota_f[:].unsqueeze(1).to_broadcast([P, cnt, P]),
                    op=mybir.AluOpType.is_equal,
                )
            return S

        class GatherStream:
            """Emits MC-chunk dma_gather calls lazily over one parity's chunks.

            tab is a strided parity view [[2*d, n], [1, d]] of the full table;
            elem_step = 2*d skips the other parity's rows."""

            def __init__(self, pool, tab, nchunks, idx_base, d_in):
                self.pool, self.tab = pool, tab
                self.n, self.base, self.d = nchunks, idx_base, d_in
                self.bufs = []

            def ensure(self, upto):
                while len(self.bufs) * MC < min(upto, self.n):
                    k = len(self.bufs)
                    ncall = min(MC, self.n - k * MC)
                    g = self.pool.tile([P, MC, self.d], BF16, tag="g")
                    ci = self.base + k * MC
                    nc.gpsimd.dma_gather(
                        g[:, :ncall, :], self.tab,
                        idx_t[:, ci * 8:(ci + ncall) * 8],
                        ncall * P, ncall * P, self.d,
                        elem_step=2 * self.d,
                    )
                    self.bufs.append(g)

            def chunk(self, cc):
                return self.bufs[cc // MC], cc % MC

        def parity_views(tab_full):
            v = tab_full[:].rearrange("(n two) d -> n two d", two=2)
            return v[:, 0, :], v[:, 1, :]

        def agg_block(t, j, S, glo, ghi):
            """One feature block of the aggregate: PSUM [P, P] over all chunks."""
            ncl, nchh = int(nch[0][t]), int(nch[1][t])
            nch_t = ncl + nchh
            ps = psA.tile([P, P], F32, tag="agg")
            done = 0
            for (stream, cbase, cnt, soff) in ((glo, int(lo0[t]), ncl, 0),
                                               (ghi, int(hi0[t]), nchh, ncl)):
                for cc in range(cnt):
                    g, slot = stream.chunk(cbase + cc)
                    nc.tensor.matmul(
                        ps[:],
                        lhsT=g[:, slot, j * P:(j + 1) * P],
                        rhs=S[:, soff + cc, :],
                        start=(done == 0), stop=(done == nch_t - 1),
                    )
                    done += 1
            assert done == nch_t
            return ps

        def bn_finalize(layer, stats_all, nbo, n_sb):
            """Global BN scale/bias from per-(tile,block) bn_stats.

            stats_all: [P, nbo, ntiles, 6]. Returns (sc, bi) each [P, nbo]."""
            stg = smp.tile([P, 2 * nbo], F32, tag=f"stg{layer}")
            for j in range(nbo):
                nc.vector.bn_aggr(out=stg[:, 2 * j:2 * j + 2],
                                  in_=stats_all[:, j, :, :])
            nc.sync.dma_start(out=st_sh[layer][:], in_=stg[:])
            nc.gpsimd.collective_compute(
                "AllGather", mybir.AluOpType.bypass,
                ins=[st_sh[layer].opt()], outs=[st_full[layer].opt()],
                replica_groups=rg)
            stall = smp.tile([P, n_cores, nbo, 2], F32, tag=f"stall{layer}")
            nc.sync.dma_start(
                out=stall[:], in_=st_full[layer][:].rearrange("(c p) s -> p c s", p=P))
            sm = stall[:, :, :, 0:1]
            sv = stall[:, :, :, 1:2]
            q8 = smp.tile([P, n_cores, nbo], F32, tag="q8")
            nc.vector.tensor_mul(out=q8[:].unsqueeze(3), in0=sm, in1=sm)
            nc.vector.tensor_add(out=q8[:].unsqueeze(3), in0=q8[:].unsqueeze(3), in1=sv)
            m8 = smp.tile([P, n_cores, nbo], F32, tag="m8")
            nc.vector.tensor_copy(out=m8[:].unsqueeze(3), in_=sm)
            half = n_cores
            while half > 1:
                half //= 2
                for buf in (m8, q8):
                    nc.vector.tensor_add(out=buf[:, :half, :],
                                         in0=buf[:, :half, :],
                                         in1=buf[:, half:2 * half, :])
            Em = smp.tile([P, nbo], F32, tag="Em")
            nc.scalar.mul(Em[:], m8[:, 0, :], 1.0 / n_cores)
            Eq = smp.tile([P, nbo], F32, tag="Eq")
            nc.scalar.mul(Eq[:], q8[:, 0, :], 1.0 / n_cores)
            var = smp.tile([P, nbo], F32, tag="var")
            nc.vector.tensor_mul(out=var[:], in0=Em[:], in1=Em[:])
            nc.vector.tensor_tensor(out=var[:], in0=Eq[:], in1=var[:],
                                    op=mybir.AluOpType.subtract)
            rs = smp.tile([P, nbo], F32, tag="rs")
            nc.scalar.activation(out=rs[:], in_=var[:],
                                 func=mybir.ActivationFunctionType.Sqrt,
                                 bias=eps_t[:], scale=1.0)
            nc.vector.reciprocal(out=rs[:], in_=rs[:])
            gt = smp.tile([P, nbo], F32, tag="gt")
            nc.sync.dma_start(out=gt[:], in_=g_d[layer][:].rearrange("(j p) -> p j", p=P))
            bt = smp.tile([P, nbo], F32, tag="bt")
            nc.sync.dma_start(out=bt[:], in_=b_d[layer][:].rearrange("(j p) -> p j", p=P))
            sc = n_sb.tile([P, nbo], F32, tag=f"sc{layer}", name=f"sc{layer}")
            nc.vector.tensor_mul(out=sc[:], in0=gt[:], in1=rs[:])
            bi = n_sb.tile([P, nbo], F32, tag=f"bi{layer}", name=f"bi{layer}")
            nc.vector.tensor_mul(out=bi[:], in0=Em[:], in1=sc[:])
            nc.vector.tensor_tensor(out=bi[:], in0=bt[:], in1=bi[:],
                                    op=mybir.AluOpType.subtract)
            return sc, bi

        def bn_apply(store, sc, bi, nbo):
            for j in range(nbo):
                nc.scalar.activation(
                    out=store[j][:],
                    in_=store[j][:],
                    func=mybir.ActivationFunctionType.Relu,
                    bias=bi[:, j:j + 1], scale=sc[:, j:j + 1])

        def z_compute(h_blocks, wl, d_out, z_sh, z_full):
            """z = h @ Wl per tile, then one AllGather."""
            nbk = len(h_blocks)
            for t in range(ntiles):
                w = tw(t)
                pz = psC.tile([P, max(d_out, 1)], F32, tag="z")
                for k in range(nbk):
                    nc.tensor.matmul(pz[:w, :], lhsT=h_blocks[k][:, t * P:t * P + w],
                                     rhs=wl[:, k * d_out:(k + 1) * d_out],
                                     start=(k == 0), stop=(k == nbk - 1))
                zs = wp.tile([P, d_out], BF16, tag="zs")
                nc.scalar.copy(out=zs[:w, :], in_=pz[:w, :])
                nc.sync.dma_start(out=z_sh[t * P:t * P + w, :], in_=zs[:w, :])
            nc.gpsimd.collective_compute(
                "AllGather", mybir.AluOpType.bypass,
                ins=[z_sh.opt()], outs=[z_full.opt()], replica_groups=rg)

        def layer_gather_agg(d_in, tabA, tabB, nbo, consume):
            """Per-tile gather + aggregate + consume(t, j, agg, rdb)."""
            glo = GatherStream(gplo, tabA, nlo, 0, d_in)
            ghi = GatherStream(gphi, tabB, nhi, nlo, d_in)
            for t in range(ntiles):
                glo.ensure(int(lo0[t]) + int(nch[0][t]))
                ghi.ensure(int(hi0[t]) + int(nch[1][t]))
                S = build_onehot(t)
                rdb = rd_bcast(t)
                for j in range(nbo):
                    agg = agg_block(t, j, S, glo, ghi)
                    consume(t, j, agg, rdb)

        # =============== LAYER 0 ===============
        hA = [hAB.tile([P, ntiles * P], BF16, tag=f"hA{j}", name=f"hA{j}") for j in range(nb1)]
        hB = [hAB.tile([P, ntiles * P], BF16, tag=f"hB{j}", name=f"hB{j}") for j in range(nb2)]

        l0_cm = tc.tile_pool(name="l0", bufs=1)
        l0p = l0_cm.__enter__()
        xoT = l0p.tile([P, shard], BF16)
        nc.sync.dma_start(out=xoT[:], in_=x_own_T[:])
        wl0 = l0p.tile([P, d1], F32)
        nc.sync.dma_start(out=wl0[:], in_=Wl0[:])
        wr0 = l0p.tile([P, d1], BF16)
        nc.sync.dma_start(out=wr0[:], in_=Wr0[:])
        stats0 = l0p.tile([P, nb1, ntiles, 6], F32)

        def consume0(t, _j, agg, rdb):
            w = tw(t)
            mean0 = wp.tile([P, P], F32, tag="mean")
            nc.vector.tensor_mul(out=mean0[:], in0=agg[:], in1=rdb[:])
            for j in range(nb1):
                ph = psB.tile([P, P], F32, tag="mm")
                nc.tensor.matmul(ph[:, :w], lhsT=wl0[:, j * P:(j + 1) * P],
                                 rhs=mean0[:, :w], start=True, stop=False)
                nc.tensor.matmul(ph[:, :w], lhsT=wr0[:, j * P:(j + 1) * P],
                                 rhs=xoT[:, t * P:t * P + w], start=False, stop=True)
                nc.vector.bn_stats(out=stats0[:, j, t, :], in_=ph[:, :w])
                nc.scalar.copy(out=hA[j][:, t * P:t * P + w], in_=ph[:, :w])

        xg_ev, xg_od = parity_views(xg)
        layer_gather_agg(d0, xg_ev, xg_od, 1, consume0)

        sc0, bi0 = bn_finalize(0, stats0, nb1, pp)
        bn_apply(hA, sc0, bi0, nb1)   # hA now holds h1 (bf16)
        l0_cm.__exit__(None, None, None)

        # =============== z1 + AllGather ===============
        z_compute(hA, wl1, d2, z1_sh, z1_full)

        # =============== LAYER 1 ===============
        l1_cm = tc.tile_pool(name="l1", bufs=1)
        l1p = l1_cm.__enter__()
        stats1 = l1p.tile([P, nb2, ntiles, 6], F32)

        # Wr side for ALL tiles first: independent of the AllGather, fills it
        pwst1 = [l1p.tile([P, ntiles * P], BF16, tag=f"pw1_{j}", name=f"pw1_{j}")
                 for j in range(nb2)]
        for t in range(ntiles):
            w = tw(t)
            for j in range(nb2):
                pw = psB.tile([P, P], F32, tag="mm")
                for k in range(d1 // P):
                    nc.tensor.matmul(
                        pw[:, :w],
                        lhsT=wr1[:, k * d2 + j * P:k * d2 + (j + 1) * P],
                        rhs=hA[k][:, t * P:t * P + w],
                        start=(k == 0), stop=(k == d1 // P - 1))
                nc.scalar.copy(out=pwst1[j][:, t * P:t * P + w], in_=pw[:, :w])

        def consume1(t, j, agg, rdb):
            w = tw(t)
            mean1 = wp.tile([P, P], F32, tag="mean")
            nc.vector.tensor_mul(out=mean1[:], in0=agg[:], in1=rdb[:])
            raw = wp.tile([P, P], F32, tag="raw")
            nc.vector.tensor_add(out=raw[:, :w], in0=mean1[:, :w],
                                 in1=pwst1[j][:, t * P:t * P + w])
            nc.vector.bn_stats(out=stats1[:, j, t, :], in_=raw[:, :w])
            nc.scalar.copy(out=hB[j][:, t * P:t * P + w], in_=raw[:, :w])

        z1_ev, z1_od = parity_views(z1_full)
        layer_gather_agg(d2, z1_ev, z1_od, nb2, consume1)

        sc1, bi1 = bn_finalize(1, stats1, nb2, pp)
        bn_apply(hB, sc1, bi1, nb2)   # hB = h2 (bf16)
        l1_cm.__exit__(None, None, None)

        # =============== z2 + AllGather ===============
        z_compute(hB, wl2, d3, z2_sh, z2_full)

        # =============== LAYER 2 ===============
        l2_cm = tc.tile_pool(name="l2", bufs=1)
        l2p = l2_cm.__enter__()
        rawC = [l2p.tile([P, ntiles * P], BF16, tag=f"rawC{j}", name=f"rawC{j}") for j in range(nb3)]
        stats2 = l2p.tile([P, nb3, ntiles, 6], F32)

        pwst2 = [l2p.tile([P, ntiles * P], BF16, tag=f"pw2_{j}", name=f"pw2_{j}")
                 for j in range(nb3)]
        for t in range(ntiles):
            w = tw(t)
            for j in range(nb3):
                pw = psB.tile([P, P], F32, tag="mm")
                for k in range(d2 // P):
                    nc.tensor.matmul(
                        pw[:, :w],
                        lhsT=wr2[:, k * d3 + j * P:k * d3 + (j + 1) * P],
                        rhs=hB[k][:, t * P:t * P + w],
                        start=(k == 0), stop=(k == d2 // P - 1))
                nc.scalar.copy(out=pwst2[j][:, t * P:t * P + w], in_=pw[:, :w])

        def consume2(t, j, agg, rdb):
            w = tw(t)
            mean2 = wp.tile([P, P], F32, tag="mean")
            nc.vector.tensor_mul(out=mean2[:], in0=agg[:], in1=rdb[:])
            raw = wp.tile([P, P], F32, tag="raw")
            nc.vector.tensor_add(out=raw[:, :w], in0=mean2[:, :w],
                                 in1=pwst2[j][:, t * P:t * P + w])
            nc.vector.bn_stats(out=stats2[:, j, t, :], in_=raw[:, :w])
            nc.scalar.copy(out=rawC[j][:, t * P:t * P + w], in_=raw[:, :w])

        z2_ev, z2_od = parity_views(z2_full)
        layer_gather_agg(d3, z2_ev, z2_od, nb3, consume2)

        sc2, bi2 = bn_finalize(2, stats2, nb3, pp)

        # BN+ReLU fused into the f32 upcast, then transpose to node-major
        for t in range(ntiles):
            w = tw(t)
            for j in range(nb3):
                ap = wp.tile([P, P], F32, tag="bnap")
                nc.scalar.activation(
                    out=ap[:, :w], in_=rawC[j][:, t * P:t * P + w],
                    func=mybir.ActivationFunctionType.Relu,
                    bias=bi2[:, j:j + 1], scale=sc2[:, j:j + 1])
                pt = psC.tile([P, 2 * P], F32, tag="z")
                nc.tensor.transpose(out=pt[:, :P], in_=ap[:],
                                    identity=ident[:])
                ot = wp.tile([P, P], F32, tag="ot")
                nc.scalar.copy(out=ot[:w, :], in_=pt[:w, :P])
                nc.sync.dma_start(out=yout[t * P:t * P + w, j * P:(j + 1) * P],
                                    in_=ot[:w, :])

        l2_cm.__exit__(None, None, None)

    nc.compile()
    return nc


# ---------------------------------------------------------------- top level
def make_in_maps(x, edge_index, weights, meta, per_core):
    """weights: dict with Wl0..Wl2, Wr0..Wr2, g0..g2, b0..b2 (numpy fp32)."""
    n_cores, shard = meta["n_cores"], meta["shard"]
    bf = lambda a: np.asarray(a, dtype=ml_dtypes.bfloat16)
    f32 = lambda a: np.ascontiguousarray(np.asarray(a, dtype=np.float32))
    x = np.asarray(x, dtype=np.float32)
    shared = {
        "xg": bf(x),
        "Wl0": f32(weights["Wl0"]), "Wr0": bf(weights["Wr0"]),
        "Wl1": bf(weights["Wl1"]), "Wr1": bf(weights["Wr1"]),
        "Wl2": bf(weights["Wl2"]), "Wr2": bf(weights["Wr2"]),
        "gn0": f32(weights["g0"]), "bn0": f32(weights["b0"]),
        "gn1": f32(weights["g1"]), "bn1": f32(weights["b1"]),
        "gn2": f32(weights["g2"]), "bn2": f32(weights["b2"]),
    }
    in_maps = []
    for c in range(n_cores):
        m = dict(shared)
        m["x_own_T"] = bf(np.ascontiguousarray(x[c * shard:(c + 1) * shard].T))
        m["idx16"] = per_core[c]["idx16"]
        m["dstl"] = per_core[c]["dstl"]
        m["rd"] = per_core[c]["rd"]
        in_maps.append(m)
    return in_maps


# ============================================================ entry point
_N_NODES = 50000
_DIMS = [128, 512, 256, 128]
_N_CORES = 8
_EPS = 1e-5


def kernel(x, edge_index, Wl0, bl0, Wr0, g0, b0, Wl1, bl1, Wr1, g1, b1,
           Wl2, bl2, Wr2, g2, b2):
    """Full-input GraphSAGE forward on 8 trn2 NeuronCores. bl* cancel under
    BatchNorm and are unused."""
    from concourse.bass_utils import run_bass_kernel_spmd
    x = np.asarray(x, dtype=np.float32)
    edge_index = np.asarray(edge_index)
    meta, per_core = preprocess(edge_index, _N_NODES, _N_CORES)
    nc = build_kernel(meta, _DIMS, eps=_EPS)
    weights = {
        "Wl0": np.asarray(Wl0), "Wr0": np.asarray(Wr0),
        "Wl1": np.asarray(Wl1), "Wr1": np.asarray(Wr1),
        "Wl2": np.asarray(Wl2), "Wr2": np.asarray(Wr2),
        "g0": np.asarray(g0), "b0": np.asarray(b0),
        "g1": np.asarray(g1), "b1": np.asarray(b1),
        "g2": np.asarray(g2), "b2": np.asarray(b2),
    }
    in_maps = make_in_maps(x, edge_index, weights, meta, per_core)
    res = run_bass_kernel_spmd(nc, in_maps, list(range(_N_CORES)))
    out = np.concatenate([res.results[c]["yout"] for c in range(_N_CORES)], axis=0)
    return out.astype(np.float32)
